# revision 42
# baseline (speedup 1.0000x reference)
"""Fused ReLU + 4x RMSNorm + 3x (matmul + residual-add) kernel for TRN2.

Reference computation (per token row t, hidden dim H=2048):
    x1 = relu(x); resid = x1
    for s in 0..2:
        y = rmsnorm(resid, g_s)                # norm over H
        resid = y @ W_s + resid
    out = rmsnorm(resid, g3)

Sharding: pure data-parallel over the token dim (32768 tokens -> 8 cores x
4096 tokens). Each row's computation is independent, so no collectives are
needed; W/g are replicated per core.

Key restructure vs the straightforward mapping: the gain g_s is folded into
the weights host-side (W'_s = diag(g_s) @ W_s, bf16), and the per-token
rsqrt factor rs_s is applied at PSUM-consume time:

    p_s = bf16(resid_s) @ W'_s          # raw residual is the matmul operand
    resid_{s+1} = resid_s + rs_s * p_s  # rs folded into the accumulate

so the TensorE stationary operand (the DMA-xbar-transposed resid tile) only
needs a bf16 cast after each residual slice finalizes -- the norm reduction
chain (reduce/sqrt/recip) is off the transpose critical path and only gates
the psum consume, which trails the matmuls by a full 16-matmul group.

Per-core pipeline (SPMD, same program on all 8 cores), blocks of TB=512
tokens (nt=4 tiles of 128):
  - phase s: for each of nb=4 output column groups, 4 token tiles x 16
    chained matmuls accumulate [128,512] PSUM tiles; consume = one DVE
    scalar_tensor_tensor (resid = psum*rs + resid), an ACT bf16 cast of
    the slice, and one DVE scalar_tensor_tensor square-with-accum for the
    next norm's sum of squares. At the last column each tile's xbar
    transpose is issued, so the next phase's stationary operands are
    ready early. (tensor_tensor_reduce faults on HW; ACT squares are
    ~160us slower on HW — both rejected experimentally.)
  - Queue segregation: boundary transposes on the ACT HWDGE queue,
    produce0 transposes on SP; all bulk DMA (W in 4x512KB chunks, x
    loads, out stores) on gpsimd/SWDGE so their DMASW completion lanes
    never entangle the HWDGE lanes the critical transposes wait on.
  - W tiles prefetch 4 column-groups ahead (bufs=5 so the WAR reaches an
    already-executed column); x loads for block b+1 land in phase 1,
    produce0 compute in phase 2, output stores for block b-1 in phase 0,
    all away from the n=3 boundary column.
"""

import sys

import numpy as np

try:
    import concourse.bass as bass  # noqa: F401
except ImportError:  # pragma: no cover
    sys.path.insert(0, "/opt/trn_rl_repo")

import concourse.bass as bass
import concourse.tile as tile
from concourse import bacc, mybir
from concourse.bass_utils import run_bass_kernel_spmd

import ml_dtypes

EPS = 1e-6
TOKENS = 32768
HIDDEN = 2048
N_CORES = 8
T_CORE = TOKENS // N_CORES  # 4096
TB = 512  # tokens per block
KC = HIDDEN // 128  # 16 contraction chunks
NB = HIDDEN // 512  # 4 output column groups
F32 = mybir.dt.float32
BF16 = mybir.dt.bfloat16

# Feature toggles (bisect aids; the fast path enables all)
USE_STT = True       # fused (psum*rs)+resid via DVE scalar_tensor_tensor
USE_TTR = False      # fused square+reduce via DVE tensor_tensor_reduce
SP_TRANSPOSE = True  # produce0 transposes on the SP queue (else ACT)
SWDGE_BULK = True    # W/x/out DMAs on gpsimd (Pool/SWDGE) (else SP)
STT_SQUARE = True    # square+accum via DVE scalar_tensor_tensor (else ACT)


def build_program(t_core=T_CORE, hidden=HIDDEN, tb=TB, reps=1):
    """Build the per-core Bass program (SPMD: identical on all cores).
    reps>1 wraps the whole pipeline in a hardware For_i loop that recomputes
    the same output; used only for slope-based device timing."""
    nt = tb // 128          # token tiles per block
    nblk = t_core // tb     # blocks
    kc = hidden // 128      # contraction chunks
    nb = hidden // 512      # output column blocks
    assert tb % 128 == 0 and t_core % tb == 0 and hidden % 512 == 0

    nc = bacc.Bacc("TRN2", target_bir_lowering=False, debug=False)

    x_d = nc.dram_tensor("x", [t_core, hidden], F32, kind="ExternalInput").ap()
    w_d = [
        nc.dram_tensor(f"W{i}", [nb, 128, kc, 512], BF16, kind="ExternalInput").ap()
        for i in range(3)
    ]
    g3_d = nc.dram_tensor("g3", [hidden], F32, kind="ExternalInput").ap()
    out_d = nc.dram_tensor("out", [t_core, hidden], F32, kind="ExternalOutput").ap()

    add = mybir.AluOpType.add
    mult = mybir.AluOpType.mult
    sqrt = mybir.ActivationFunctionType.Sqrt
    square = mybir.ActivationFunctionType.Square
    copyf = mybir.ActivationFunctionType.Copy

    with tile.TileContext(nc) as tc:
        with (
            tc.tile_pool(name="const", bufs=1) as const_pool,
            tc.tile_pool(name="resid", bufs=2) as resid_pool,
            tc.tile_pool(name="xb", bufs=4) as xb_pool,
            tc.tile_pool(name="yT", bufs=2) as yt_pool,
            tc.tile_pool(name="w", bufs=5) as w_pool,
            tc.tile_pool(name="scrb", bufs=2) as scrb_pool,
            tc.tile_pool(name="small", bufs=24) as small_pool,
            tc.tile_pool(name="psum", bufs=8, space="PSUM") as psum_pool,
        ):
            eps_t = const_pool.tile([128, 1], F32)
            nc.vector.memset(eps_t, EPS)

            def bcast(ap):
                return bass.AP(
                    tensor=ap.tensor, offset=ap.offset, ap=[[0, 128]] + list(ap.ap)
                )

            g3t = const_pool.tile([128, hidden], F32, tag="g3")
            nc.gpsimd.dma_start(out=g3t, in_=bcast(g3_d))

            # ---- per-block pipeline state ----
            # state[blk] = dict with resid tile, per-stage ssp/rs/xb/yt refs.
            state = {}

            # Global W-load schedule: tiles (s, n) consumed in order per
            # block; each load is emitted a couple of column-groups ahead.
            wload_seq = [(s, n) for s in range(3) for n in range(nb)]

            bulk = nc.gpsimd if SWDGE_BULK else nc.sync

            def emit_wload(blk, s, n):
                # SWDGE (Pool) lanes + 4-chunk split: bulk W traffic stays off
                # the HWDGE semaphore lanes used by the latency-critical
                # transposes, and no single transfer blocks the DMA pipe long.
                wt = w_pool.tile([128, kc, 512], BF16, tag="w",
                                 name=f"w{blk}_{s}_{n}")
                for c in range(4):
                    bulk.dma_start(
                        out=wt[:, c * (kc // 4) : (c + 1) * (kc // 4), :],
                        in_=w_d[s][n][:, c * (kc // 4) : (c + 1) * (kc // 4), :],
                    )
                state[blk]["w"][(s, n)] = wt

            def wload_ahead(blk, idx):
                """Emit the W load `idx` positions into blk's schedule,
                rolling into the next block when idx >= 12."""
                b, i = blk, idx
                while i >= len(wload_seq):
                    b, i = b + 1, i - len(wload_seq)
                if b >= nblk:
                    return
                s, n = wload_seq[i]
                if (s, n) not in state[b]["w"]:
                    emit_wload(b, s, n)

            def new_block_state(blk):
                resid = resid_pool.tile(
                    [128, nt, hidden], F32, tag="resid", name=f"resid{blk}"
                )
                state[blk] = {
                    "resid": resid,
                    "ssp": {},   # (stage) -> [per-m tiles]
                    "rs": {},    # (stage) -> [per-m tiles]
                    "xb": {},    # (stage) -> [per-m tiles]
                    "yt": {},    # (stage) -> yT tile
                    "w": {},     # (s, n) -> tile
                }

            def alloc_ssp(blk, s):
                state[blk]["ssp"][s] = [
                    small_pool.tile([128, nb], F32, tag=f"ssp{m}",
                                    name=f"ssp_b{blk}_s{s}_{m}")
                    for m in range(nt)
                ]

            def emit_square(scrb, rsl, accum_ap):
                """sum-of-squares of a resid slice into one ssp column."""
                if STT_SQUARE:
                    nc.vector.scalar_tensor_tensor(
                        out=scrb, in0=rsl, scalar=1.0, in1=rsl,
                        op0=mult, op1=mult, accum_out=accum_ap,
                    )
                else:
                    nc.scalar.activation(
                        out=scrb, in_=rsl, func=square, accum_out=accum_ap,
                    )

            def rs_chain(blk, s, m):
                """rs = 1/sqrt(mean(ssp)+eps) for (stage s, tile m)."""
                ssp = state[blk]["ssp"][s][m]
                ss = small_pool.tile([128, 1], F32, tag="ss",
                                     name=f"ss{blk}_{s}_{m}")
                rs = small_pool.tile([128, 1], F32, tag="rs",
                                     name=f"rs{blk}_{s}_{m}")
                nc.vector.tensor_reduce(ss, ssp, axis=mybir.AxisListType.X, op=add)
                nc.scalar.activation(
                    out=rs, in_=ss, func=sqrt, bias=eps_t[:, :], scale=1.0 / hidden
                )
                nc.vector.reciprocal(rs, rs)
                state[blk]["rs"].setdefault(s, {})[m] = rs

            def produce0_load(blk, m):
                """x load for one 128-token tile (issued ~a phase before the
                compute part, so the DMA is long done when relu runs)."""
                resid = state[blk]["resid"]
                bulk.dma_start(
                    out=resid[:, m, :],
                    in_=x_d[blk * tb + m * 128 : blk * tb + (m + 1) * 128, :],
                )

            def produce0_tile(blk, m):
                """relu + bf16 cast + squares + transpose for one 128-token
                tile of block blk (stage-0 operand prep)."""
                st = state[blk]
                resid = st["resid"]
                # relu + bf16 cast on DVE: keeps the ACT stream (which owns
                # the latency-critical transposes) free of x-load-dependent
                # work, so coalesced semaphore waits can't couple them.
                nc.vector.tensor_scalar_max(resid[:, m, :], resid[:, m, :], 0.0)
                xb = xb_pool.tile([128, hidden], BF16, tag="xb",
                                  name=f"xb{blk}_0_{m}")
                st["xb"].setdefault(0, {})[m] = xb
                nc.vector.tensor_copy(xb, resid[:, m, :])
                # transpose before the squares: the stationary operand is the
                # critical path, the squares only feed rs (slack ~1 phase).
                # SP queue: keeps these off the ACT sequencer, whose in-order
                # stream carries the tighter phase-boundary transposes.
                (nc.sync if SP_TRANSPOSE else nc.scalar).dma_start_transpose(
                    st["yt"][0][:, m * kc : (m + 1) * kc, :], xb
                )
                ssp = st["ssp"][0][m]
                for n in range(nb):
                    rsl = resid[:, m, n * 512 : (n + 1) * 512]
                    scrb = scrb_pool.tile([128, 512], BF16, tag="sq",
                                          name=f"sq{blk}_0_{m}_{n}")
                    emit_square(scrb, rsl, ssp[:, n : n + 1])
                rs_chain(blk, 0, m)

            def mm_phase(blk, s, interleave=None):
                """resid += rs_s * (bf16(resid) @ W'_s); prep stage s+1
                operands. `interleave`: callback(col_n) emitting unrelated
                work (stage3 stores / next-block produce) between column
                groups."""
                st = state[blk]
                resid = st["resid"]
                yt = st["yt"][s]
                alloc_ssp(blk, s + 1)
                nssp = st["ssp"][s + 1]
                boundary = s < 2
                if boundary:
                    nyt = yt_pool.tile(
                        [128, nt * kc, 128], BF16, tag="yT",
                        name=f"yt{blk}_{s + 1}"
                    )
                    st["yt"][s + 1] = nyt
                    for m in range(nt):
                        xb = xb_pool.tile([128, hidden], BF16, tag="xb",
                                          name=f"xb{blk}_{s + 1}_{m}")
                        st["xb"].setdefault(s + 1, {})[m] = xb
                for n in range(nb):
                    wt = st["w"][(s, n)]
                    for m in range(nt):
                        ps = psum_pool.tile(
                            [128, 512], F32, tag="ps",
                            name=f"ps{blk}_{s}_{n}_{m}"
                        )
                        for k in range(kc):
                            nc.tensor.matmul(
                                ps,
                                yt[:, m * kc + k, :],
                                wt[:, k, :],
                                start=(k == 0),
                                stop=(k == kc - 1),
                            )
                        rs = st["rs"][s][m]
                        rslice = resid[:, m, n * 512 : (n + 1) * 512]
                        # fused consume: resid = (psum * rs) + resid
                        if USE_STT:
                            nc.vector.scalar_tensor_tensor(
                                out=rslice, in0=ps, scalar=rs[:, :], in1=rslice,
                                op0=mult, op1=add,
                            )
                        else:
                            scr = scrb_pool.tile([128, 512], F32, tag="scrf",
                                                 name=f"scrf{blk}_{s}_{n}_{m}")
                            nc.scalar.activation(out=scr, in_=ps, func=copyf,
                                                 scale=rs[:, :])
                            nc.vector.tensor_add(rslice, rslice, scr)
                        if boundary:
                            # non-boundary casts on DVE (ACT ops are
                            # disproportionately expensive on HW); the n=3
                            # cast stays on ACT so the boundary chain
                            # cast->transpose needs no extra engine hop.
                            xb = st["xb"][s + 1][m]
                            xsl = xb[:, n * 512 : (n + 1) * 512]
                            if n == nb - 1:
                                nc.scalar.activation(out=xsl, in_=rslice,
                                                     func=copyf)
                            else:
                                nc.vector.tensor_copy(xsl, rslice)
                            if n == nb - 1:
                                nc.scalar.dma_start_transpose(
                                    nyt[:, m * kc : (m + 1) * kc, :], xb
                                )
                        scrb = scrb_pool.tile([128, 512], BF16, tag="sq",
                                              name=f"sq{blk}_{s}_{n}_{m}")
                        emit_square(scrb, rslice, nssp[m][:, n : n + 1])
                        if n == nb - 1:
                            rs_chain(blk, s + 1, m)
                            if s == 2 and blk == nblk - 1:
                                # last block: drain output tiles inline
                                stage3_tile(blk, m)
                    # x/store interleave first so W prefetch can't queue-block
                    # it; with bufs=5 the W load's WAR reaches a column that
                    # already executed, so the load starts immediately.
                    if interleave is not None:
                        interleave(n)
                    wload_ahead(blk, s * nb + n + 4)

            def stage3_tile(blk, m):
                """out tile m = resid * rs3 * g3, then store."""
                st = state[blk]
                resid = st["resid"]
                rs = st["rs"][3][m]
                # fused: out = (resid * rs3) * g3
                nc.vector.scalar_tensor_tensor(
                    out=resid[:, m, :], in0=resid[:, m, :], scalar=rs[:, :],
                    in1=g3t, op0=mult, op1=mult,
                )
                nc.gpsimd.dma_start(
                    out=out_d[blk * tb + m * 128 : blk * tb + (m + 1) * 128, :],
                    in_=resid[:, m, :],
                )

            # ---- main pipeline ----
            def pipeline():
                state.clear()
                # startup: block 0 operand prep + first W loads
                new_block_state(0)
                st0 = state[0]
                st0["yt"][0] = yt_pool.tile([128, nt * kc, 128], BF16,
                                            tag="yT", name="yt0_0")
                alloc_ssp(0, 0)
                # startup: tile 0's operand chain ahead of the other bulk
                # loads so the first matmul group isn't queued behind them.
                produce0_load(0, 0)
                wload_ahead(0, 0)
                produce0_tile(0, 0)
                produce0_load(0, 1)
                produce0_tile(0, 1)
                wload_ahead(0, 1)
                produce0_load(0, 2)
                produce0_tile(0, 2)
                produce0_load(0, 3)
                produce0_tile(0, 3)
                wload_ahead(0, 2)
                wload_ahead(0, 3)

                for blk in range(nblk):
                    prev = blk - 1 if blk > 0 else None
                    nxt = blk + 1 if blk + 1 < nblk else None

                    # keep the boundary column (n=3) free of bulk DMA: the
                    # phase-tail transposes are latency-critical there.
                    def phase0_il(n, prev=prev):
                        if prev is not None:
                            for m in ({0: [0], 1: [1], 2: [2, 3]}.get(n, [])):
                                stage3_tile(prev, m)

                    mm_phase(blk, 0, interleave=phase0_il)

                    if nxt is not None:
                        new_block_state(nxt)
                        stn = state[nxt]
                        stn["yt"][0] = yt_pool.tile(
                            [128, nt * kc, 128], BF16, tag="yT",
                            name=f"yt{nxt}_0"
                        )
                        alloc_ssp(nxt, 0)

                    def phase1_il(n, nxt=nxt):
                        if nxt is not None:
                            for m in ({0: [0, 1], 1: [2], 2: [3]}.get(n, [])):
                                produce0_load(nxt, m)

                    mm_phase(blk, 1, interleave=phase1_il)

                    def phase2_il(n, nxt=nxt):
                        if nxt is not None:
                            produce0_tile(nxt, n)

                    mm_phase(blk, 2, interleave=phase2_il)
                    # free stale W refs (pool rotation handles buffers)
                    state[blk]["w"].clear()
                # (last block's stage3 tiles were drained inline in phase 2)

            if reps == 1:
                pipeline()
            else:
                with tc.For_i(0, reps, 1):
                    pipeline()

    nc.compile()
    return nc


_CACHE = {}


def _get_program(key=(T_CORE, HIDDEN, TB)):  # noqa: B008
    if key not in _CACHE:
        _CACHE[key] = build_program(*key)
    return _CACHE[key]


def make_in_maps(inputs):
    """Host-side prep shared by run() and the bench harness: shard x over
    cores, fold g_s into W_s (bf16) and pre-tile to [nb, 128, kc, 512]."""
    x = np.ascontiguousarray(np.asarray(inputs["x"], dtype=np.float32))
    ws = []
    for i in range(3):
        w = np.asarray(inputs[f"W{i}"], dtype=np.float32)
        g = np.asarray(inputs[f"g{i}"], dtype=np.float32)
        wt = (w * g[:, None]).astype(ml_dtypes.bfloat16)
        wt = np.ascontiguousarray(
            wt.reshape(KC, 128, NB, 512).transpose(2, 1, 0, 3)
        )
        ws.append(wt)
    g3 = np.ascontiguousarray(np.asarray(inputs["g3"], dtype=np.float32))

    in_maps = []
    for c in range(N_CORES):
        im = {"x": x[c * T_CORE : (c + 1) * T_CORE], "g3": g3}
        for i in range(3):
            im[f"W{i}"] = ws[i]
        in_maps.append(im)
    return in_maps


def run(inputs, trace=False):
    """Run on 8 NeuronCores. Returns (out, BassKernelResults)."""
    nc = _get_program()
    in_maps = make_in_maps(inputs)
    res = run_bass_kernel_spmd(nc, in_maps, list(range(N_CORES)), trace=trace)
    out = np.concatenate([res.results[c]["out"] for c in range(N_CORES)], axis=0)
    return out, res


def kernel(**inputs) -> np.ndarray:
    out, _ = run(inputs, trace=False)
    return out


# revision 48
# speedup vs baseline: 1.0444x; 1.0444x over previous
"""Fused ReLU + 4x RMSNorm + 3x (matmul + residual-add) kernel for TRN2.

Reference computation (per token row t, hidden dim H=2048):
    x1 = relu(x); resid = x1
    for s in 0..2:
        y = rmsnorm(resid, g_s)                # norm over H
        resid = y @ W_s + resid
    out = rmsnorm(resid, g3)

Sharding: pure data-parallel over the token dim (32768 tokens -> 8 cores x
4096 tokens). Each row's computation is independent, so no collectives are
needed; W/g are replicated per core.

Key restructure vs the straightforward mapping: the gain g_s is folded into
the weights host-side (W'_s = diag(g_s) @ W_s, bf16), and the per-token
rsqrt factor rs_s is applied at PSUM-consume time:

    p_s = bf16(resid_s) @ W'_s          # raw residual is the matmul operand
    resid_{s+1} = resid_s + rs_s * p_s  # rs folded into the accumulate

so the TensorE stationary operand (the DMA-xbar-transposed resid tile) only
needs a bf16 cast after each residual slice finalizes -- the norm reduction
chain (reduce/sqrt/recip) is off the transpose critical path and only gates
the psum consume, which trails the matmuls by a full 16-matmul group.

Per-core pipeline (SPMD, same program on all 8 cores), blocks of TB=512
tokens (nt=4 tiles of 128):
  - phase s: paired tile loop — tiles (0,1) sweep all nb=4 column groups,
    then tiles (2,3) — 16 chained matmuls per [128,512] PSUM tile;
    consume = one DVE scalar_tensor_tensor (resid = psum*rs + resid), an
    ACT bf16 cast of the slice, and one DVE scalar_tensor_tensor
    square-with-accum for the next norm's sum of squares. Each tile's
    xbar transpose is issued at its last column, which the pairing places
    mid-phase (pair 0) or staggered at the tail (pair 1), so the next
    phase's stationary operands are always ready with slack.
    (tensor_tensor_reduce faults on HW; ACT squares are ~160us slower on
    HW — both rejected experimentally.)
  - Queue segregation: boundary transposes on the ACT HWDGE queue,
    produce0 transposes on SP; all bulk DMA (W in 4x512KB chunks, x
    loads, out stores) on gpsimd/SWDGE so their DMASW completion lanes
    never entangle the HWDGE lanes the critical transposes wait on.
  - W tiles prefetch 4 column-groups ahead (bufs=5 so the WAR reaches an
    already-executed column); x loads for block b+1 land in phase 1,
    produce0 compute in phase 2, output stores for block b-1 in phase 0,
    all away from the n=3 boundary column.
"""

import sys

import numpy as np

try:
    import concourse.bass as bass  # noqa: F401
except ImportError:  # pragma: no cover
    sys.path.insert(0, "/opt/trn_rl_repo")

import concourse.bass as bass
import concourse.tile as tile
from concourse import bacc, mybir
from concourse.bass_utils import run_bass_kernel_spmd

import ml_dtypes

EPS = 1e-6
TOKENS = 32768
HIDDEN = 2048
N_CORES = 8
T_CORE = TOKENS // N_CORES  # 4096
TB = 512  # tokens per block
KC = HIDDEN // 128  # 16 contraction chunks
NB = HIDDEN // 512  # 4 output column groups
F32 = mybir.dt.float32
BF16 = mybir.dt.bfloat16

# Feature toggles (bisect aids; the fast path enables all)
USE_STT = True       # fused (psum*rs)+resid via DVE scalar_tensor_tensor
USE_TTR = False      # fused square+reduce via DVE tensor_tensor_reduce
SP_TRANSPOSE = True  # produce0 transposes on the SP queue (else ACT)
SWDGE_BULK = True    # W/x/out DMAs on gpsimd (Pool/SWDGE) (else SP)
STT_SQUARE = True    # square+accum via DVE scalar_tensor_tensor (else ACT)


def build_program(t_core=T_CORE, hidden=HIDDEN, tb=TB, reps=1):
    """Build the per-core Bass program (SPMD: identical on all cores).
    reps>1 wraps the whole pipeline in a hardware For_i loop that recomputes
    the same output; used only for slope-based device timing."""
    nt = tb // 128          # token tiles per block
    nblk = t_core // tb     # blocks
    kc = hidden // 128      # contraction chunks
    nb = hidden // 512      # output column blocks
    assert tb % 128 == 0 and t_core % tb == 0 and hidden % 512 == 0

    nc = bacc.Bacc("TRN2", target_bir_lowering=False, debug=False)

    x_d = nc.dram_tensor("x", [t_core, hidden], F32, kind="ExternalInput").ap()
    w_d = [
        nc.dram_tensor(f"W{i}", [nb, 128, kc, 512], BF16, kind="ExternalInput").ap()
        for i in range(3)
    ]
    g3_d = nc.dram_tensor("g3", [hidden], F32, kind="ExternalInput").ap()
    out_d = nc.dram_tensor("out", [t_core, hidden], F32, kind="ExternalOutput").ap()

    add = mybir.AluOpType.add
    mult = mybir.AluOpType.mult
    sqrt = mybir.ActivationFunctionType.Sqrt
    square = mybir.ActivationFunctionType.Square
    copyf = mybir.ActivationFunctionType.Copy

    with tile.TileContext(nc) as tc:
        with (
            tc.tile_pool(name="const", bufs=1) as const_pool,
            tc.tile_pool(name="resid", bufs=2) as resid_pool,
            tc.tile_pool(name="xb", bufs=4) as xb_pool,
            tc.tile_pool(name="yT", bufs=2) as yt_pool,
            tc.tile_pool(name="w", bufs=5) as w_pool,
            tc.tile_pool(name="scrb", bufs=2) as scrb_pool,
            tc.tile_pool(name="small", bufs=24) as small_pool,
            tc.tile_pool(name="psum", bufs=8, space="PSUM") as psum_pool,
        ):
            eps_t = const_pool.tile([128, 1], F32)
            nc.vector.memset(eps_t, EPS)

            def bcast(ap):
                return bass.AP(
                    tensor=ap.tensor, offset=ap.offset, ap=[[0, 128]] + list(ap.ap)
                )

            g3t = const_pool.tile([128, hidden], F32, tag="g3")
            nc.gpsimd.dma_start(out=g3t, in_=bcast(g3_d))

            # ---- per-block pipeline state ----
            # state[blk] = dict with resid tile, per-stage ssp/rs/xb/yt refs.
            state = {}

            # Global W-load schedule: tiles (s, n) consumed in order per
            # block; each load is emitted a couple of column-groups ahead.
            wload_seq = [(s, n) for s in range(3) for n in range(nb)]

            bulk = nc.gpsimd if SWDGE_BULK else nc.sync

            def emit_wload(blk, s, n):
                # SWDGE (Pool) lanes + 4-chunk split: bulk W traffic stays off
                # the HWDGE semaphore lanes used by the latency-critical
                # transposes, and no single transfer blocks the DMA pipe long.
                wt = w_pool.tile([128, kc, 512], BF16, tag="w",
                                 name=f"w{blk}_{s}_{n}")
                for c in range(4):
                    bulk.dma_start(
                        out=wt[:, c * (kc // 4) : (c + 1) * (kc // 4), :],
                        in_=w_d[s][n][:, c * (kc // 4) : (c + 1) * (kc // 4), :],
                    )
                state[blk]["w"][(s, n)] = wt

            def wload_ahead(blk, idx):
                """Emit the W load `idx` positions into blk's schedule,
                rolling into the next block when idx >= 12."""
                b, i = blk, idx
                while i >= len(wload_seq):
                    b, i = b + 1, i - len(wload_seq)
                if b >= nblk:
                    return
                s, n = wload_seq[i]
                if (s, n) not in state[b]["w"]:
                    emit_wload(b, s, n)

            def new_block_state(blk):
                resid = resid_pool.tile(
                    [128, nt, hidden], F32, tag="resid", name=f"resid{blk}"
                )
                state[blk] = {
                    "resid": resid,
                    "ssp": {},   # (stage) -> [per-m tiles]
                    "rs": {},    # (stage) -> [per-m tiles]
                    "xb": {},    # (stage) -> [per-m tiles]
                    "yt": {},    # (stage) -> yT tile
                    "w": {},     # (s, n) -> tile
                }

            def alloc_ssp(blk, s):
                state[blk]["ssp"][s] = [
                    small_pool.tile([128, nb], F32, tag=f"ssp{m}",
                                    name=f"ssp_b{blk}_s{s}_{m}")
                    for m in range(nt)
                ]

            def emit_square(scrb, rsl, accum_ap):
                """sum-of-squares of a resid slice into one ssp column."""
                if STT_SQUARE:
                    nc.vector.scalar_tensor_tensor(
                        out=scrb, in0=rsl, scalar=1.0, in1=rsl,
                        op0=mult, op1=mult, accum_out=accum_ap,
                    )
                else:
                    nc.scalar.activation(
                        out=scrb, in_=rsl, func=square, accum_out=accum_ap,
                    )

            def rs_chain(blk, s, m):
                """rs = 1/sqrt(mean(ssp)+eps) for (stage s, tile m)."""
                ssp = state[blk]["ssp"][s][m]
                ss = small_pool.tile([128, 1], F32, tag="ss",
                                     name=f"ss{blk}_{s}_{m}")
                rs = small_pool.tile([128, 1], F32, tag="rs",
                                     name=f"rs{blk}_{s}_{m}")
                nc.vector.tensor_reduce(ss, ssp, axis=mybir.AxisListType.X, op=add)
                nc.scalar.activation(
                    out=rs, in_=ss, func=sqrt, bias=eps_t[:, :], scale=1.0 / hidden
                )
                nc.vector.reciprocal(rs, rs)
                state[blk]["rs"].setdefault(s, {})[m] = rs

            def produce0_load(blk, m):
                """x load for one 128-token tile (issued ~a phase before the
                compute part, so the DMA is long done when relu runs)."""
                resid = state[blk]["resid"]
                bulk.dma_start(
                    out=resid[:, m, :],
                    in_=x_d[blk * tb + m * 128 : blk * tb + (m + 1) * 128, :],
                )

            def produce0_tile(blk, m):
                """relu + bf16 cast + squares + transpose for one 128-token
                tile of block blk (stage-0 operand prep)."""
                st = state[blk]
                resid = st["resid"]
                # relu + bf16 cast on DVE: keeps the ACT stream (which owns
                # the latency-critical transposes) free of x-load-dependent
                # work, so coalesced semaphore waits can't couple them.
                nc.vector.tensor_scalar_max(resid[:, m, :], resid[:, m, :], 0.0)
                xb = xb_pool.tile([128, hidden], BF16, tag="xb",
                                  name=f"xb{blk}_0_{m}")
                st["xb"].setdefault(0, {})[m] = xb
                nc.vector.tensor_copy(xb, resid[:, m, :])
                # transpose before the squares: the stationary operand is the
                # critical path, the squares only feed rs (slack ~1 phase).
                # SP queue: keeps these off the ACT sequencer, whose in-order
                # stream carries the tighter phase-boundary transposes.
                (nc.sync if SP_TRANSPOSE else nc.scalar).dma_start_transpose(
                    st["yt"][0][:, m * kc : (m + 1) * kc, :], xb
                )
                ssp = st["ssp"][0][m]
                for n in range(nb):
                    rsl = resid[:, m, n * 512 : (n + 1) * 512]
                    scrb = scrb_pool.tile([128, 512], BF16, tag="sq",
                                          name=f"sq{blk}_0_{m}_{n}")
                    emit_square(scrb, rsl, ssp[:, n : n + 1])
                rs_chain(blk, 0, m)

            def mm_phase(blk, s, interleave=None):
                """resid += rs_s * (bf16(resid) @ W'_s); prep stage s+1
                operands. `interleave`: callback(col_n) emitting unrelated
                work (stage3 stores / next-block produce) between column
                groups."""
                st = state[blk]
                resid = st["resid"]
                yt = st["yt"][s]
                alloc_ssp(blk, s + 1)
                nssp = st["ssp"][s + 1]
                boundary = s < 2
                if boundary:
                    nyt = yt_pool.tile(
                        [128, nt * kc, 128], BF16, tag="yT",
                        name=f"yt{blk}_{s + 1}"
                    )
                    st["yt"][s + 1] = nyt
                    for m in range(nt):
                        xb = xb_pool.tile([128, hidden], BF16, tag="xb",
                                          name=f"xb{blk}_{s + 1}_{m}")
                        st["xb"].setdefault(s + 1, {})[m] = xb
                # paired tile loop: tiles (0,1) sweep all columns first,
                # then (2,3); each tile's final cast + transpose lands mid-
                # phase (pair 0) or staggered at the tail (pair 1).
                for half in range(2):
                    tiles = (0, 1) if half == 0 else (2, 3)
                    for n in range(nb):
                        wt = st["w"][(s, n)]
                        for m in tiles:
                            ps = psum_pool.tile(
                                [128, 512], F32, tag="ps",
                                name=f"ps{blk}_{s}_{n}_{m}"
                            )
                            for k in range(kc):
                                nc.tensor.matmul(
                                    ps,
                                    yt[:, m * kc + k, :],
                                    wt[:, k, :],
                                    start=(k == 0),
                                    stop=(k == kc - 1),
                                )
                            rs = st["rs"][s][m]
                            rslice = resid[:, m, n * 512 : (n + 1) * 512]
                            # fused consume: resid = (psum * rs) + resid
                            nc.vector.scalar_tensor_tensor(
                                out=rslice, in0=ps, scalar=rs[:, :], in1=rslice,
                                op0=mult, op1=add,
                            )
                            if boundary:
                                xb = st["xb"][s + 1][m]
                                nc.scalar.activation(
                                    out=xb[:, n * 512 : (n + 1) * 512],
                                    in_=rslice, func=copyf,
                                )
                                if n == nb - 1:
                                    nc.scalar.dma_start_transpose(
                                        nyt[:, m * kc : (m + 1) * kc, :], xb
                                    )
                            scrb = scrb_pool.tile([128, 512], BF16, tag="sq",
                                                  name=f"sq{blk}_{s}_{n}_{m}")
                            emit_square(scrb, rslice, nssp[m][:, n : n + 1])
                            if n == nb - 1:
                                rs_chain(blk, s + 1, m)
                                if s == 2 and blk == nblk - 1:
                                    # last block: drain output tiles inline
                                    stage3_tile(blk, m)
                        if half == 0:
                            if interleave is not None:
                                interleave(n)
                        else:
                            wload_ahead(blk, s * nb + n + 4)

            def stage3_tile(blk, m):
                """out tile m = resid * rs3 * g3, then store."""
                st = state[blk]
                resid = st["resid"]
                rs = st["rs"][3][m]
                # fused: out = (resid * rs3) * g3
                nc.vector.scalar_tensor_tensor(
                    out=resid[:, m, :], in0=resid[:, m, :], scalar=rs[:, :],
                    in1=g3t, op0=mult, op1=mult,
                )
                nc.gpsimd.dma_start(
                    out=out_d[blk * tb + m * 128 : blk * tb + (m + 1) * 128, :],
                    in_=resid[:, m, :],
                )

            # ---- main pipeline ----
            def pipeline():
                state.clear()
                # startup: block 0 operand prep + first W loads
                new_block_state(0)
                st0 = state[0]
                st0["yt"][0] = yt_pool.tile([128, nt * kc, 128], BF16,
                                            tag="yT", name="yt0_0")
                alloc_ssp(0, 0)
                # startup: tile 0's operand chain ahead of the other bulk
                # loads so the first matmul group isn't queued behind them.
                produce0_load(0, 0)
                wload_ahead(0, 0)
                produce0_tile(0, 0)
                produce0_load(0, 1)
                produce0_tile(0, 1)
                wload_ahead(0, 1)
                produce0_load(0, 2)
                produce0_tile(0, 2)
                produce0_load(0, 3)
                produce0_tile(0, 3)
                wload_ahead(0, 2)
                wload_ahead(0, 3)

                for blk in range(nblk):
                    prev = blk - 1 if blk > 0 else None
                    nxt = blk + 1 if blk + 1 < nblk else None

                    # keep the boundary column (n=3) free of bulk DMA: the
                    # phase-tail transposes are latency-critical there.
                    def phase0_il(n, prev=prev):
                        if prev is not None:
                            for m in ({0: [0], 1: [1], 2: [2, 3]}.get(n, [])):
                                stage3_tile(prev, m)

                    mm_phase(blk, 0, interleave=phase0_il)

                    if nxt is not None:
                        new_block_state(nxt)
                        stn = state[nxt]
                        stn["yt"][0] = yt_pool.tile(
                            [128, nt * kc, 128], BF16, tag="yT",
                            name=f"yt{nxt}_0"
                        )
                        alloc_ssp(nxt, 0)

                    def phase1_il(n, nxt=nxt):
                        if nxt is not None:
                            for m in ({0: [0, 1], 1: [2], 2: [3]}.get(n, [])):
                                produce0_load(nxt, m)

                    mm_phase(blk, 1, interleave=phase1_il)

                    def phase2_il(n, nxt=nxt):
                        if nxt is not None:
                            produce0_tile(nxt, n)

                    mm_phase(blk, 2, interleave=phase2_il)
                    # free stale W refs (pool rotation handles buffers)
                    state[blk]["w"].clear()
                # (last block's stage3 tiles were drained inline in phase 2)

            if reps == 1:
                pipeline()
            else:
                with tc.For_i(0, reps, 1):
                    pipeline()

    nc.compile()
    return nc


_CACHE = {}


def _get_program(key=(T_CORE, HIDDEN, TB)):  # noqa: B008
    if key not in _CACHE:
        _CACHE[key] = build_program(*key)
    return _CACHE[key]


def make_in_maps(inputs):
    """Host-side prep shared by run() and the bench harness: shard x over
    cores, fold g_s into W_s (bf16) and pre-tile to [nb, 128, kc, 512]."""
    x = np.ascontiguousarray(np.asarray(inputs["x"], dtype=np.float32))
    ws = []
    for i in range(3):
        w = np.asarray(inputs[f"W{i}"], dtype=np.float32)
        g = np.asarray(inputs[f"g{i}"], dtype=np.float32)
        wt = (w * g[:, None]).astype(ml_dtypes.bfloat16)
        wt = np.ascontiguousarray(
            wt.reshape(KC, 128, NB, 512).transpose(2, 1, 0, 3)
        )
        ws.append(wt)
    g3 = np.ascontiguousarray(np.asarray(inputs["g3"], dtype=np.float32))

    in_maps = []
    for c in range(N_CORES):
        im = {"x": x[c * T_CORE : (c + 1) * T_CORE], "g3": g3}
        for i in range(3):
            im[f"W{i}"] = ws[i]
        in_maps.append(im)
    return in_maps


def run(inputs, trace=False):
    """Run on 8 NeuronCores. Returns (out, BassKernelResults)."""
    nc = _get_program()
    in_maps = make_in_maps(inputs)
    res = run_bass_kernel_spmd(nc, in_maps, list(range(N_CORES)), trace=trace)
    out = np.concatenate([res.results[c]["out"] for c in range(N_CORES)], axis=0)
    return out, res


def kernel(**inputs) -> np.ndarray:
    out, _ = run(inputs, trace=False)
    return out
